# revision 10
# baseline (speedup 1.0000x reference)
"""DeepseekV2 MLA attention (prefill, causal) on 8 trn2 NeuronCores. v2.

Math: non-absorbed form (k_nope = ckv @ w_uk, v = ckv @ w_uv per head),
scores over d=192, causal softmax without max-subtraction (|score|*scale
< 8 for this operator family), denominators via an all-ones matmul over a
DVE-accumulated exp-sum.

Sharding: heads tensor-parallel (2 heads/core); shared projections
(q_a, ckv, k_pe) sequence-sharded then AllGathered in TWO collectives
(ckv+k_pe early, q_a later) so transfer hides under projection compute.
Attention outputs (feature-major oT per head) are AllToAll'd back to
sequence shards; each core computes its own 256-row slice of
y = o @ w_o against the FULL w_o (prefetched during attention) - no
ReduceScatter of [S,E] partials.

Precision: all matmul operands bf16 (host-converted weights/x); PSUM
accumulation, softmax statistics, rmsnorm statistics and the final y
stay fp32.  Scores' rope part is packed: the two 64-contraction matmuls
(one per head) run concurrently on disjoint PE row-halves via
tile_position.
"""
import sys

sys.path.insert(0, "/opt/trn_rl_repo")

import ml_dtypes
import numpy as np

import concourse.bass as bass
from concourse import bacc
import concourse.mybir as mybir
import concourse.tile as tile
from concourse.bass_utils import run_bass_kernel_spmd

F32 = mybir.dt.float32
F32R = mybir.dt.float32r
BF = mybir.dt.bfloat16
AF = mybir.ActivationFunctionType
MUL = mybir.AluOpType.mult
BF_NP = ml_dtypes.bfloat16

B, S, E, H = 1, 2048, 2048, 16
DN, DR, DV, R, QLR = 128, 64, 128, 512, 1536
EPS = 1e-6
NCORES = 8
SL = S // NCORES          # 256 sequence rows per core
HPC = H // NCORES         # 2 heads per core
SM_SCALE = (DN + DR) ** -0.5
NEG = -1e30
ROPE_BASE = 10000.0

QKC = E // 128            # 16 contraction chunks over E
QRC = QLR // 128          # 12 row chunks of q_a
CRC = R // 128            # 4 row chunks of ckv
NQC = S // 512            # 4 query column chunks
NKT = S // 128            # 16 key tiles
AG1R = R + DR             # 576 rows in the early allgather


def _rope_tables():
    inv_freq = 1.0 / (ROPE_BASE ** (np.arange(0, DR, 2, dtype=np.float64) / DR))
    ang = np.arange(S, dtype=np.float64)[:, None] * inv_freq[None, :]
    cos = np.concatenate([np.cos(ang), np.cos(ang)], -1).astype(np.float32)  # [S,DR]
    sin = np.concatenate([np.sin(ang), np.sin(ang)], -1).astype(np.float32)
    return cos.T.copy(), sin.T.copy()  # [DR, S] feature-major


def _consts():
    # rot(v)[j] = -v[j+32] for j<32 ; v[j-32] for 32<=j<64, as lhsT[k,m]
    p = np.zeros((64, 64), dtype=np.float32)
    for j in range(32):
        p[j + 32, j] = -1.0
    for j in range(32, 64):
        p[j - 32, j] = 1.0
    prot = np.zeros((128, 128), dtype=np.float32)
    prot[:64, :64] = p
    prot[64:, 64:] = p
    cosT, sinT = _rope_tables()
    cos2 = np.concatenate([cosT, cosT], 0)  # [128, S] (two stacked heads)
    sin2 = np.concatenate([sinT, sinT], 0)
    # boundary masks for scoresT tiles [k 128 | q 512]; m = kt - 4*qc
    ii = np.arange(128)[:, None]
    jj = np.arange(512)[None, :]
    masks = np.stack(
        [np.where(jj - ii - 128 * m >= 0, 0.0, NEG).astype(np.float32) for m in range(4)]
    )
    return prot, cos2, sin2, masks


def _build(skip_collectives=False):
    nc = bacc.Bacc(None, num_devices=NCORES)

    x_sl = nc.dram_tensor("x_sl", [SL, E], BF, kind="ExternalInput")
    w_q_a = nc.dram_tensor("w_q_a", [E, QLR], BF, kind="ExternalInput")
    w_kv_a = nc.dram_tensor("w_kv_a", [E, R + DR], BF, kind="ExternalInput")
    lnw_q = nc.dram_tensor("lnw_q", [QLR, 1], F32, kind="ExternalInput")
    lnw_kv = nc.dram_tensor("lnw_kv", [R, 1], F32, kind="ExternalInput")
    w_qb_sl = nc.dram_tensor("w_qb_sl", [QLR, 2 * DN + 2 * DR], BF, kind="ExternalInput")
    w_uk_sl = nc.dram_tensor("w_uk_sl", [R, 2 * DN], BF, kind="ExternalInput")
    w_uv_sl = nc.dram_tensor("w_uv_sl", [R, 2 * DV], BF, kind="ExternalInput")
    w_o_full = nc.dram_tensor("w_o_full", [H * DV, E], BF, kind="ExternalInput")
    cos_sl = nc.dram_tensor("cos_sl", [DR, SL], F32, kind="ExternalInput")
    sin_sl = nc.dram_tensor("sin_sl", [DR, SL], F32, kind="ExternalInput")
    ident_in = nc.dram_tensor("ident_in", [128, 128], BF, kind="ExternalInput")
    ones_in = nc.dram_tensor("ones_in", [128, 128], F32R, kind="ExternalInput")
    prot_in = nc.dram_tensor("prot_in", [128, 128], BF, kind="ExternalInput")
    y_sl = nc.dram_tensor("y_sl", [SL, E], F32, kind="ExternalOutput")

    _, cos2_np, sin2_np, masks_np = _consts()
    cos2_t = nc.inline_tensor(cos2_np, name="cos2_c")
    sin2_t = nc.inline_tensor(sin2_np, name="sin2_c")
    masks_t = nc.inline_tensor(masks_np, name="masks_c")

    ag1_in = nc.dram_tensor("ag1_in", [AG1R, SL], BF)
    ag1_out = nc.dram_tensor("ag1_out", [NCORES * AG1R, SL], BF, addr_space="Shared")
    ag2_in = nc.dram_tensor("ag2_in", [QLR, SL], BF)
    ag2_out = nc.dram_tensor("ag2_out", [NCORES * QLR, SL], BF, addr_space="Shared")
    a2a_in = nc.dram_tensor("a2a_in", [H * DV, SL], BF)
    a2a_out = nc.dram_tensor("a2a_out", [H * DV, SL], BF)

    def collect(kind, in_t, out_t):
        if skip_collectives:
            if kind == "AllGather":
                rows = in_t.shape[0]
                for c in range(NCORES):
                    nc.gpsimd.dma_start(
                        out=out_t[c * rows:(c + 1) * rows, :], in_=in_t[:, :])
            else:
                nc.gpsimd.dma_start(out=out_t[:, :], in_=in_t[:, :])
        else:
            nc.gpsimd.collective_compute(
                kind, mybir.AluOpType.bypass,
                replica_groups=[list(range(NCORES))],
                ins=[in_t[:, :].opt()], outs=[out_t[:, :].opt()])

    with tile.TileContext(nc) as tc:
        with tc.tile_pool(name="consts", bufs=1) as cp:
            ident_sb = cp.tile([128, 128], BF)
            nc.sync.dma_start(out=ident_sb, in_=ident_in[:, :])
            ones_sb = cp.tile([128, 128], F32R)
            nc.scalar.dma_start(out=ones_sb, in_=ones_in[:, :])
            prot_sb = cp.tile([128, 128], BF)
            nc.scalar.dma_start(out=prot_sb, in_=prot_in[:, :])
            eps_sb = cp.tile([128, 1], F32)
            nc.vector.memset(eps_sb[:], EPS)
            lnwq_sb = cp.tile([128, QRC], F32)
            nc.scalar.dma_start(
                out=lnwq_sb, in_=lnw_q.rearrange("(rc p) one -> p rc one", p=128))
            lnwkv_sb = cp.tile([128, CRC], F32)
            nc.scalar.dma_start(
                out=lnwkv_sb, in_=lnw_kv.rearrange("(rc p) one -> p rc one", p=128))
            cos2_sb = cp.tile([128, S], F32)
            nc.scalar.dma_start(out=cos2_sb, in_=cos2_t[:, :])
            sin2_sb = cp.tile([128, S], F32)
            nc.scalar.dma_start(out=sin2_sb, in_=sin2_t[:, :])
            mask_sb = cp.tile([128, 4, 512], F32)
            nc.scalar.dma_start(out=mask_sb, in_=masks_t.rearrange("m p f -> p m f"))

            # tiles that persist from stage B through the final y matmuls
            with tc.tile_pool(name="persist", bufs=1) as pp:
                qnT = [pp.tile([128, S], BF, tag=f"qnT{h}", name=f"qnT{h}") for h in range(HPC)]
                qpeT = pp.tile([128, S], BF, tag="qpeT")
                knT = [pp.tile([128, S], BF, tag=f"knT{h}", name=f"knT{h}") for h in range(HPC)]
                kpe2 = pp.tile([128, S], BF, tag="kpe2")
                v_sb = pp.tile([128, NKT, HPC * DV], BF, tag="v_sb")

                # ---------------- stage A: sharded q_a / ckv / k_pe -------------
                with tc.tile_pool(name="pa", bufs=1) as pa, \
                     tc.tile_pool(name="psA", bufs=2, space="PSUM") as psA:
                    xT = pa.tile([128, QKC, SL], BF, tag="xT")
                    for sc in range(SL // 128):
                        x_sb = pa.tile([128, E], BF, tag="x_sb", bufs=2)
                        nc.sync.dma_start(out=x_sb, in_=x_sl[sc * 128:(sc + 1) * 128, :])
                        for ec in range(QKC):
                            pt = psA.tile([128, 128], BF, tag="pt", bufs=2)
                            nc.tensor.transpose(pt[:], x_sb[:, ec * 128:(ec + 1) * 128],
                                                ident_sb[:])
                            nc.vector.tensor_copy(xT[:, ec, sc * 128:(sc + 1) * 128],
                                                  pt[:])

                    def feat_major_block(w_sb, col_off, ncols_chunks, lnw_sb, nfeat,
                                         stage, tag):
                        """project xT -> [feature|SL] chunks, rmsnorm feature-major,
                        write bf16 into stage rows."""
                        raw = pa.tile([128, ncols_chunks, SL], F32, tag=f"raw{tag}")
                        ssq = psA.tile([128, SL], F32, tag="ssqA", bufs=1)
                        for rc in range(ncols_chunks):
                            pq = psA.tile([128, SL], F32, tag="pqA", bufs=2)
                            for kc in range(QKC):
                                nc.tensor.matmul(
                                    pq[:],
                                    w_sb[:, kc, col_off + rc * 128:
                                         col_off + (rc + 1) * 128],
                                    xT[:, kc, :], start=(kc == 0), stop=(kc == QKC - 1))
                            nc.vector.tensor_copy(raw[:, rc, :], pq[:])
                            sq = pa.tile([128, SL], F32R, tag=f"sq{tag}", bufs=2)
                            nc.scalar.activation(out=sq, in_=pq[:], func=AF.Square)
                            nc.tensor.matmul(ssq[:], ones_sb[:], sq[:],
                                             start=(rc == 0),
                                             stop=(rc == ncols_chunks - 1))
                        rstd = pa.tile([128, SL], F32, tag=f"rstd{tag}")
                        nc.scalar.activation(out=rstd, in_=ssq[:], func=AF.Sqrt,
                                             scale=1.0 / nfeat, bias=eps_sb[:])
                        nc.vector.reciprocal_approx_fast(out=rstd[:], in_=rstd[:])
                        for rc in range(ncols_chunks):
                            nc.vector.scalar_tensor_tensor(
                                out=stage[:, rc, :], in0=raw[:, rc, :],
                                scalar=lnw_sb[:, rc:rc + 1], in1=rstd[:],
                                op0=MUL, op1=MUL)

                    # --- kv path first: its (small) allgather fires early ---
                    wkv_sb = pa.tile([128, QKC, R + DR], BF, tag="wkv")
                    wkvv = w_kv_a.rearrange("(kc p) m -> p kc m", p=128)
                    for kc in range(QKC):
                        nc.sync.dma_start(out=wkv_sb[:, kc, :], in_=wkvv[:, kc, :])
                    ag1_stage = pa.tile([128, CRC, SL], BF, tag="ag1s")
                    feat_major_block(wkv_sb, 0, CRC, lnwkv_sb, R, ag1_stage, "c")
                    # k_pe chunk [64 | SL] + rope (fp32 math, bf16 out)
                    ppe = psA.tile([64, SL], F32, tag="ppe", bufs=1)
                    for kc in range(QKC):
                        nc.tensor.matmul(ppe[:], wkv_sb[:, kc, R:R + DR], xT[:, kc, :],
                                         start=(kc == 0), stop=(kc == QKC - 1))
                    kpe_f = pa.tile([64, SL], F32, tag="kpe_f")
                    nc.scalar.copy(kpe_f[:], ppe[:])
                    kpe_b = pa.tile([64, SL], BF, tag="kpe_b")
                    nc.vector.tensor_copy(kpe_b[:], kpe_f[:])
                    prot_ps = psA.tile([64, SL], F32, tag="prot_ps", bufs=1)
                    nc.tensor.matmul(prot_ps[:], prot_sb[0:64, 0:64], kpe_b[:],
                                     start=True, stop=True)
                    cos_sb = pa.tile([64, SL], F32, tag="cos_sb")
                    nc.scalar.dma_start(out=cos_sb, in_=cos_sl[:, :])
                    sin_sb = pa.tile([64, SL], F32, tag="sin_sb")
                    nc.scalar.dma_start(out=sin_sb, in_=sin_sl[:, :])
                    t1 = pa.tile([64, SL], F32, tag="t1")
                    nc.vector.tensor_mul(t1[:], kpe_f[:], cos_sb[:])
                    t2 = pa.tile([64, SL], F32, tag="t2")
                    nc.vector.tensor_mul(t2[:], prot_ps[:], sin_sb[:])
                    kpe_out = pa.tile([64, SL], BF, tag="kpe_out")
                    nc.vector.tensor_add(kpe_out[:], t1[:], t2[:])
                    nc.gpsimd.dma_start(
                        out=ag1_in[0:R, :].rearrange("(rc p) s -> p rc s", p=128),
                        in_=ag1_stage[:])
                    nc.gpsimd.dma_start(out=ag1_in[R:AG1R, :], in_=kpe_out[:])
                    collect("AllGather", ag1_in, ag1_out)

                    # --- q path ---
                    wqa_sb = pa.tile([128, QKC, QLR], BF, tag="wqa")
                    wqav = w_q_a.rearrange("(kc p) m -> p kc m", p=128)
                    for kc in range(QKC):
                        nc.sync.dma_start(out=wqa_sb[:, kc, :], in_=wqav[:, kc, :])
                    ag2_stage = pa.tile([128, QRC, SL], BF, tag="ag2s")
                    feat_major_block(wqa_sb, 0, QRC, lnwq_sb, QLR, ag2_stage, "q")
                    nc.gpsimd.dma_start(
                        out=ag2_in[:, :].rearrange("(rc p) s -> p rc s", p=128),
                        in_=ag2_stage[:])
                    collect("AllGather", ag2_in, ag2_out)

                agv1 = ag1_out.rearrange("(c r) s -> r c s", c=NCORES)
                agv2 = ag2_out.rearrange("(c r) s -> r c s", c=NCORES)

                # w_o lives from its stage-B prefetch through stage D; opened
                # only after stage A's big transients are freed
                lp_pool = tc.tile_pool(name="late", bufs=1)
                lp = lp_pool.__enter__()
                wo_sb = lp.tile([128, QKC, E], BF, tag="wo_sb")

                # ---------------- stage B: k_nopeT, v, qT (+rope) ----------------
                with tc.tile_pool(name="pb", bufs=1) as pb, \
                     tc.tile_pool(name="psB", bufs=2, space="PSUM") as psB:
                    wqb_sb = pb.tile([128, QRC, 2 * DN + 2 * DR], BF, tag="wqb")
                    wqbv = w_qb_sl.rearrange("(kc p) m -> p kc m", p=128)
                    for kc in range(QRC):
                        nc.sync.dma_start(out=wqb_sb[:, kc, :], in_=wqbv[:, kc, :])
                    wuk_sb = pb.tile([128, CRC, 2 * DN], BF, tag="wuk")
                    nc.sync.dma_start(
                        out=wuk_sb, in_=w_uk_sl.rearrange("(rc p) m -> p rc m", p=128))
                    wuv_sb = pb.tile([128, CRC, 2 * DV], BF, tag="wuv")
                    nc.sync.dma_start(
                        out=wuv_sb, in_=w_uv_sl.rearrange("(rc p) m -> p rc m", p=128))
                    # w_o prefetch on the ACT HWDGE ring; consumed only in stage D
                    wov = w_o_full.rearrange("(kc p) e -> p kc e", p=128)
                    for kc in range(QKC):
                        nc.scalar.dma_start(out=wo_sb[:, kc, :], in_=wov[:, kc, :])

                    ckvT = pb.tile([128, CRC, S], BF, tag="ckvT")
                    for rc in range(CRC):
                        nc.sync.dma_start(
                            out=ckvT[:, rc, :],
                            in_=agv1[rc * 128:(rc + 1) * 128, :, :])
                    nc.sync.dma_start(out=kpe2[0:64, :], in_=agv1[R:AG1R, :, :])
                    nc.sync.dma_start(out=kpe2[64:128, :], in_=agv1[R:AG1R, :, :])

                    for h in range(HPC):
                        for nq in range(NQC):
                            pk = psB.tile([128, 512], F32, tag="pk", bufs=2)
                            for rc in range(CRC):
                                nc.tensor.matmul(
                                    pk[:], wuk_sb[:, rc, h * DN:(h + 1) * DN],
                                    ckvT[:, rc, nq * 512:(nq + 1) * 512],
                                    start=(rc == 0), stop=(rc == CRC - 1))
                            nc.vector.tensor_copy(knT[h][:, nq * 512:(nq + 1) * 512],
                                                  pk[:])
                    for kt in range(NKT):
                        pv = psB.tile([128, HPC * DV], F32, tag="pv", bufs=2)
                        for rc in range(CRC):
                            nc.tensor.matmul(
                                pv[:], ckvT[:, rc, kt * 128:(kt + 1) * 128],
                                wuv_sb[:, rc, :], start=(rc == 0), stop=(rc == CRC - 1))
                        nc.vector.tensor_copy(v_sb[:, kt, :], pv[:])

                    for qc in range(NQC):
                        cs = slice(qc * 512, (qc + 1) * 512)
                        pqs = [psB.tile([128, 512], F32, tag=f"pqb{mc}", bufs=1,
                                        name=f"pqb{mc}")
                               for mc in range(3)]
                        for kc in range(QRC):
                            t = pb.tile([128, 512], BF, tag="qa_rhs", bufs=4)
                            nc.sync.dma_start(
                                out=t,
                                in_=agv2[kc * 128:(kc + 1) * 128, 2 * qc:2 * qc + 2, :])
                            for mc in range(3):
                                nc.tensor.matmul(
                                    pqs[mc][:], wqb_sb[:, kc, mc * 128:(mc + 1) * 128],
                                    t[:], start=(kc == 0), stop=(kc == QRC - 1))
                        for mc in range(HPC):
                            nc.vector.tensor_copy(qnT[mc][:, cs], pqs[mc][:])
                        qpe_raw = pb.tile([128, 512], BF, tag="qpe_raw", bufs=2)
                        nc.vector.tensor_copy(qpe_raw[:], pqs[2][:])
                        rot_ps = psB.tile([128, 512], F32, tag="rot_ps", bufs=1)
                        nc.tensor.matmul(rot_ps[:], prot_sb[:], qpe_raw[:],
                                         start=True, stop=True)
                        tq1 = pb.tile([128, 512], F32, tag="tq1", bufs=2)
                        nc.vector.tensor_mul(tq1[:], pqs[2][:], cos2_sb[:, cs])
                        tq2 = pb.tile([128, 512], F32, tag="tq2", bufs=2)
                        nc.vector.tensor_mul(tq2[:], rot_ps[:], sin2_sb[:, cs])
                        nc.vector.tensor_add(qpeT[:, cs], tq1[:], tq2[:])

                # ------------- stage C: attention, oT per head -> a2a_in ---------
                with tc.tile_pool(name="pcl", bufs=1) as pcl, \
                     tc.tile_pool(name="psC", bufs=1, space="PSUM") as psC:
                    for qc in range(NQC):
                        cs = slice(qc * 512, (qc + 1) * 512)
                        nkt = 4 * qc + 4
                        po = [psC.tile([128, 512], F32, tag=f"po{h}", bufs=1,
                                       name=f"po{h}")
                              for h in range(HPC)]
                        etsum = [pcl.tile([128, 512], F32R, tag=f"etsum{h}", bufs=1,
                                          name=f"etsum{h}")
                                 for h in range(HPC)]
                        prev = None
                        for kt in range(nkt):
                            ks = slice(kt * 128, (kt + 1) * 128)
                            m = kt - 4 * qc
                            pss = []
                            for h in range(HPC):
                                ps = psC.tile([128, 512], F32, tag=f"ps{h}", bufs=2)
                                nc.tensor.matmul(ps[:], knT[h][:, ks], qnT[h][:, cs],
                                                 start=True, stop=False)
                                nc.tensor.matmul(
                                    ps[:], kpe2[h * 64:(h + 1) * 64, ks],
                                    qpeT[h * 64:(h + 1) * 64, cs],
                                    start=False, stop=True,
                                    tile_position=(h * 64, 0))
                                pss.append(ps)
                            # po for the previous tile goes to PE *after* this
                            # tile's scores so PE never waits on the exp chain
                            if prev is not None:
                                pkt, pets = prev
                                for h in range(HPC):
                                    nc.tensor.matmul(
                                        po[h][:], v_sb[:, pkt, h * DV:(h + 1) * DV],
                                        pets[h][:], start=(pkt == 0), stop=False)
                            ets = []
                            for h in range(HPC):
                                if m >= 0:
                                    nc.vector.tensor_add(pss[h][:], pss[h][:],
                                                         mask_sb[:, m, :])
                                et = pcl.tile([128, 512], BF, tag=f"et{h}", bufs=3)
                                nc.scalar.activation(out=et, in_=pss[h][:], func=AF.Exp,
                                                     scale=SM_SCALE)
                                if kt == 0:
                                    nc.vector.tensor_copy(etsum[h][:], et[:])
                                else:
                                    nc.vector.tensor_add(etsum[h][:], etsum[h][:],
                                                         et[:])
                                ets.append(et)
                            prev = (kt, ets)
                        pkt, pets = prev
                        for h in range(HPC):
                            nc.tensor.matmul(po[h][:],
                                             v_sb[:, pkt, h * DV:(h + 1) * DV],
                                             pets[h][:],
                                             start=(pkt == 0), stop=True)
                        for h in range(HPC):
                            pdn = psC.tile([128, 512], F32, tag=f"pdn{h}", bufs=1)
                            nc.tensor.matmul(pdn[:], ones_sb[:], etsum[h][:],
                                             start=True, stop=True)
                            rec = pcl.tile([128, 512], F32, tag="rec", bufs=2)
                            nc.vector.reciprocal_approx_fast(out=rec[:], in_=pdn[:])
                            ofin = pcl.tile([128, 512], BF, tag=f"ofin{h}", bufs=2)
                            nc.vector.tensor_mul(ofin[:], po[h][:], rec[:])
                            for half in range(2):
                                dst = 2 * qc + half
                                nc.gpsimd.dma_start(
                                    out=a2a_in[dst * HPC * DV + h * 128:
                                               dst * HPC * DV + (h + 1) * 128, :],
                                    in_=ofin[:, half * 256:(half + 1) * 256])

                collect("AllToAll", a2a_in, a2a_out)

                # ------------- stage D: y slice = oT_all.T @ w_o -----------------
                with tc.tile_pool(name="pd", bufs=1) as pd, \
                     tc.tile_pool(name="psD", bufs=2, space="PSUM") as psD:
                    a2a_sb = pd.tile([128, QKC, SL], BF, tag="a2a_sb")
                    nc.scalar.dma_start(
                        out=a2a_sb,
                        in_=a2a_out.rearrange("(kc p) s -> p kc s", p=128))
                    for mc in range(SL // 128):
                        for nq in range(NQC):
                            py = psD.tile([128, 512], F32, tag="py", bufs=2)
                            for kc in range(QKC):
                                nc.tensor.matmul(
                                    py[:], a2a_sb[:, kc, mc * 128:(mc + 1) * 128],
                                    wo_sb[:, kc, nq * 512:(nq + 1) * 512],
                                    start=(kc == 0), stop=(kc == QKC - 1))
                            y_sb = pd.tile([128, 512], F32, tag="y_sb", bufs=3)
                            nc.vector.tensor_copy(y_sb[:], py[:])
                            nc.gpsimd.dma_start(
                                out=y_sl[mc * 128:(mc + 1) * 128,
                                         nq * 512:(nq + 1) * 512],
                                in_=y_sb[:])
                lp_pool.__exit__(None, None, None)
    nc.finalize()
    return nc


_NC_CACHE = None


def _get_nc():
    global _NC_CACHE
    if _NC_CACHE is None:
        _NC_CACHE = _build()
    return _NC_CACHE


def _make_in_maps(x, w_q_a, q_a_ln_w, w_q_b, w_kv_a, kv_a_ln_w, w_kv_b, w_o):
    bf = lambda a: np.ascontiguousarray(np.asarray(a, dtype=np.float32)).astype(BF_NP)
    x = np.asarray(x, dtype=np.float32)
    q_a_ln_w = np.asarray(q_a_ln_w, dtype=np.float32)
    kv_a_ln_w = np.asarray(kv_a_ln_w, dtype=np.float32)
    w_q_b = np.asarray(w_q_b, dtype=np.float32)
    w_kv_b = np.asarray(w_kv_b, dtype=np.float32)

    cosT, sinT = _rope_tables()
    wqb = w_q_b.reshape(QLR, H, DN + DR)
    wkv = w_kv_b.reshape(R, H, DN + DV)
    prot_np, _, _, _ = _consts()

    w_q_a_bf = bf(w_q_a)
    w_kv_a_bf = bf(w_kv_a)
    w_o_bf = bf(w_o)
    ident_np = np.eye(128, dtype=np.float32).astype(BF_NP)
    ones_np = np.ones((128, 128), dtype=np.float32)
    prot_bf = prot_np.astype(BF_NP)

    in_maps = []
    for c in range(NCORES):
        h0, h1 = HPC * c, HPC * c + 1
        w_qb_sl = np.concatenate(
            [wqb[:, h0, :DN], wqb[:, h1, :DN], wqb[:, h0, DN:], wqb[:, h1, DN:]],
            axis=1)
        w_uk_sl = np.concatenate([wkv[:, h0, :DN], wkv[:, h1, :DN]], axis=1)
        w_uv_sl = np.concatenate([wkv[:, h0, DN:], wkv[:, h1, DN:]], axis=1)
        in_maps.append({
            "x_sl": bf(x[0, c * SL:(c + 1) * SL, :]),
            "w_q_a": w_q_a_bf,
            "w_kv_a": w_kv_a_bf,
            "lnw_q": np.ascontiguousarray(q_a_ln_w.reshape(QLR, 1)),
            "lnw_kv": np.ascontiguousarray(kv_a_ln_w.reshape(R, 1)),
            "w_qb_sl": bf(w_qb_sl),
            "w_uk_sl": bf(w_uk_sl),
            "w_uv_sl": bf(w_uv_sl),
            "w_o_full": w_o_bf,
            "cos_sl": np.ascontiguousarray(cosT[:, c * SL:(c + 1) * SL]),
            "sin_sl": np.ascontiguousarray(sinT[:, c * SL:(c + 1) * SL]),
            "ident_in": ident_np,
            "ones_in": ones_np,
            "prot_in": prot_bf,
        })
    return in_maps


def kernel(**inputs):
    in_maps = _make_in_maps(**inputs)
    nc = _get_nc()
    # The axon terminal occasionally reports NRT_EXEC_UNIT_UNRECOVERABLE on the
    # first load after a prior session died; a retry recovers it.
    last_exc = None
    for _ in range(3):
        try:
            res = run_bass_kernel_spmd(nc, in_maps, core_ids=list(range(NCORES)))
            break
        except Exception as e:  # noqa: BLE001
            last_exc = e
    else:
        raise last_exc
    y = np.concatenate([res.results[c]["y_sl"] for c in range(NCORES)], axis=0)
    return y.reshape(B, S, E)


if __name__ == "__main__":
    nc = _build()
    print("built ok")
